# revision 19
# baseline (speedup 1.0000x reference)
"""GAT (2-layer) on 8 Trainium2 NeuronCores — streaming edge aggregation.

Strategy (graph/data parallel per the sharding hint, node-partitioned):
- Host-side staging only (index gathers / permutation / padding); ALL
  arithmetic stays on device.  Nodes are sorted by in-degree and dealt
  into 98 blocks x 1024 (128 per core x 8 cores), so each block has a
  near-uniform degree D_b (max over its 1024 nodes); per-node edge lists
  (self-loop first) are padded to D_b.
- Edge stage: per-edge source features h[src] and a_src[src] are gathered
  by the host into a contiguous slot-major stream [block][slot][d][cols]
  (pure data movement of device-computed values, like the baseline's
  _expand_a), so the device only does big sequential HWDGE DMAs — no
  dma_gather / GpSimd descriptor generation at all.
- Segment softmax+sum per block of 128 dst nodes: partition = dst slot;
  R[slot, d, :] = [ex | ex * h] after DVE add / leaky / ActE exp / DVE
  mult; then D_b identity-stationary matmuls accumulate
  psum[slot, [denom | out]] += R[:, d, :] (TensorE as a wide fp32
  accumulator; identity weights loaded once).  Softmax max-subtraction
  is skipped (logits are O(1), exp is safe).
- Node stages: weights stationary (Wcat = [W | W @ atte] built on device
  with one matmul), x^T streamed in 512-column chunks; host transposes
  between launches, so no on-device transposes anywhere.
- Head dim c-major (col = c*H + h) so the ex*h broadcast multiply is
  unit-stride innermost (DVE 2x).  bf16 storage/compute, fp32 PSUM.
- ELU(x) = max(x, exp(min(x, 0)) - 1)  (1 DVE min, 1 ActE exp, 1 DVE stt).
"""

import sys

sys.path.insert(0, "/opt/trn_rl_repo")

import numpy as np
import ml_dtypes

import concourse.bass as bass
import concourse.mybir as mybir
from concourse import bacc
from concourse.tile import TileContext
from concourse.bass_utils import run_bass_kernel_spmd

BF = ml_dtypes.bfloat16
bf16 = mybir.dt.bfloat16
f32 = mybir.dt.float32
AF = mybir.ActivationFunctionType
OP = mybir.AluOpType

N = 100000
NCORES = 8
P = 128
NBLK = 98                 # blocks per core; 98*128 = 12544 nodes/core
NODE_PAD = NBLK * P       # 12544
NTOT = NODE_PAD * NCORES  # 100352
CHUNK = 1024              # nodes per block across all cores (128 * 8)
H1, C1, F1 = 8, 16, 128   # layer-1 heads/channels; F1 = H1*C1
F2 = 64
NEG = -60000.0
GQ = 7                    # blocks per DMA group (98 = 14*7); uniform degree
NG = NBLK // GQ           # 14 groups


# ---------------------------------------------------------------- node stage
def build_node(fin, fout, extra, nodes):
    """outT = [Wcat^T @ xT] rows: [h (fout) | att terms (extra)].

    xT: [fin, nodes] shard; Wcat = [W | W @ atte] built on device.
    """
    nc = bacc.Bacc(trn_type="TRN2")
    xT = nc.declare_dram_parameter("xT", [fin, nodes], bf16, isOutput=False)
    w = nc.declare_dram_parameter("w", [fin, fout], bf16, isOutput=False)
    wt = nc.declare_dram_parameter("wt", [fout, fin], bf16, isOutput=False)
    atte = nc.declare_dram_parameter("atte", [fout, extra], bf16, isOutput=False)
    outT = nc.declare_dram_parameter(
        "outT", [fout + extra, nodes], bf16, isOutput=True
    )

    nch = (nodes + 511) // 512
    with TileContext(nc) as tc:
        with (
            tc.tile_pool(name="const", bufs=1) as cp,
            tc.tile_pool(name="sb", bufs=2) as pool,
            tc.tile_pool(name="ps", bufs=2, space="PSUM") as pp,
        ):
            # PE warmup: ~4us of dummy matmuls releases the HAM clock gate
            # (1.2 -> 2.4 GHz) while the input DMA streams in
            zt = cp.tile([P, P], bf16)
            nc.scalar.memzero(zt[:])
            wu_ps = pp.tile([P, P], f32, tag="wu")
            for _ in range(40):
                nc.tensor.matmul(
                    out=wu_ps[:], lhsT=zt[:], rhs=zt[:], start=True, stop=True
                )
            wcat = cp.tile([fin, fout + extra], bf16)
            nc.sync.dma_start(out=wcat[:, 0:fout], in_=w[:])
            wt_t = cp.tile([fout, fin], bf16)
            nc.sync.dma_start(out=wt_t[:], in_=wt[:])
            atte_t = cp.tile([fout, extra], bf16)
            nc.sync.dma_start(out=atte_t[:], in_=atte[:])
            # w_att[fi, e] = sum_hc W[fi, hc] * atte[hc, e]
            wa_ps = pp.tile([fin, extra], f32, tag="wa")
            nc.tensor.matmul(
                out=wa_ps[:], lhsT=wt_t[:], rhs=atte_t[:], start=True, stop=True
            )
            nc.vector.tensor_copy(out=wcat[:, fout : fout + extra], in_=wa_ps[:])

            xs = cp.tile([fin, nodes], bf16)
            for k, c0 in enumerate(range(0, nodes, 4096)):
                c1 = min(nodes, c0 + 4096)
                ring = nc.sync if k % 2 == 0 else nc.scalar
                ring.dma_start(out=xs[:, c0:c1], in_=xT[:, c0:c1])
            hT = cp.tile([fout, nodes], bf16)
            aT = cp.tile([extra, nodes], bf16)
            for i in range(nch):
                c0 = i * 512
                c1 = min(nodes, c0 + 512)
                h_ps = pp.tile([fout, 512], f32, tag="h")
                nc.tensor.matmul(
                    out=h_ps[:, 0 : c1 - c0],
                    lhsT=wcat[:, 0:fout],
                    rhs=xs[:, c0:c1],
                    start=True,
                    stop=True,
                )
                a_ps = pp.tile([extra, 512], f32, tag="a")
                nc.tensor.matmul(
                    out=a_ps[:, 0 : c1 - c0],
                    lhsT=wcat[:, fout : fout + extra],
                    rhs=xs[:, c0:c1],
                    start=True,
                    stop=True,
                )
                nc.scalar.copy(out=hT[:, c0:c1], in_=h_ps[:, 0 : c1 - c0])
                nc.vector.tensor_copy(out=aT[:, c0:c1], in_=a_ps[:, 0 : c1 - c0])
            nc.sync.dma_start(out=outT[0:fout, :], in_=hT[:])
            nc.scalar.dma_start(out=outT[fout : fout + extra, :], in_=aT[:])
    nc.finalize()
    return nc


# ---------------------------------------------------------------- edge stage
def build_edge(layer, dbs, gqs, dmax):
    """Edge aggregation over the core's 98 dst blocks (dbs[b] = edges/node).

    dbs is uniform within each group of GQ blocks; one DMA per group.
    Blocks are aggregated in pairs via N=2*cols matmuls into one PSUM bank.
    """
    if layer == 1:
        hh, cc, ff = H1, C1, F1      # cols: [a_src(8) | h(128)] -> 136
        cols = hh + ff
        a0 = hh                       # h starts at col 8 (16B aligned)
    else:
        hh, cc, ff = 1, F2, F2        # cols: [a_src(1) | pad(1) | h(64)] -> 66
        cols = 66
        a0 = 2
    totrows = P * int(sum(dbs))

    nc = bacc.Bacc(trn_type="TRN2")
    hgat = nc.declare_dram_parameter("hgat", [totrows, cols], bf16, isOutput=False)
    adst = nc.declare_dram_parameter("adst", [NBLK * P, hh], bf16, isOutput=False)
    ident = nc.declare_dram_parameter("ident", [P, P], bf16, isOutput=False)
    zout = nc.declare_dram_parameter("z", [NBLK * P, ff], bf16, isOutput=True)

    # tile shapes: group 0 (gq=1) carries the degree-distribution head; all
    # other groups have much smaller dg, so give them a separate (3-deep) tag
    dmax2 = max(
        (int(dbs[b]) for b, q in zip(np.cumsum([0] + list(gqs[:-1])), gqs)
         if q > 1),
        default=dmax,
    )
    with TileContext(nc) as tc:
        with (
            tc.tile_pool(name="const", bufs=1) as cp,
            tc.tile_pool(name="sbb", bufs=1) as bpool,
            tc.tile_pool(name="sb", bufs=3) as gpool,
            tc.tile_pool(name="ep", bufs=2) as pool,
            tc.tile_pool(name="ps", bufs=2, space="PSUM") as pp,
        ):
            id_t = cp.tile([P, P], bf16)
            nc.sync.dma_start(out=id_t[:], in_=ident[:])
            ad_t = cp.tile([P, NBLK, hh], bf16)
            nc.sync.dma_start(
                out=ad_t[:], in_=adst[:].rearrange("(b p) h -> p b h", p=P)
            )

            off = 0
            b0 = 0
            for g, gq in enumerate(gqs):
                dg = int(dbs[b0])
                if gq == 1:
                    Rg = bpool.tile([P, 1, dmax, cols], bf16, tag="Rb")
                else:
                    Rg = gpool.tile([P, GQ, dmax2, cols], bf16, tag="R")
                ring = nc.sync if g % 2 == 0 else nc.scalar
                ring.dma_start(
                    out=Rg[:, 0:gq, 0:dg, :],
                    in_=hgat[off : off + gq * P * dg, :].rearrange(
                        "(q p d) f -> p q d f", q=gq, p=P
                    ),
                )
                off += gq * P * dg
                # e = a_src + a_dst ; leaky_relu(0.2) ; exp
                nc.vector.tensor_tensor(
                    out=Rg[:, 0:gq, 0:dg, 0:hh],
                    in0=Rg[:, 0:gq, 0:dg, 0:hh],
                    in1=ad_t[:, b0 : b0 + gq, None, :].to_broadcast(
                        [P, gq, dg, hh]
                    ),
                    op=OP.add,
                )
                # exp(leaky_relu(e)) = max(exp(e), exp(0.2 e))  (exp monotone)
                lk = pool.tile([P, GQ, dmax, hh], bf16, tag="lk")
                nc.scalar.activation(
                    out=lk[:, 0:gq, 0:dg, :],
                    in_=Rg[:, 0:gq, 0:dg, 0:hh],
                    func=AF.Exp,
                    scale=0.2,
                )
                nc.scalar.activation(
                    out=Rg[:, 0:gq, 0:dg, 0:hh],
                    in_=Rg[:, 0:gq, 0:dg, 0:hh],
                    func=AF.Exp,
                )
                nc.vector.tensor_tensor(
                    out=Rg[:, 0:gq, 0:dg, 0:hh],
                    in0=Rg[:, 0:gq, 0:dg, 0:hh],
                    in1=lk[:, 0:gq, 0:dg, :],
                    op=OP.max,
                )
                # msg = ex * h  (unit-stride innermost on both operands;
                # per block: the ISA mem pattern allows at most 3 free dims)
                if layer == 1:
                    for q in range(gq):
                        nc.vector.tensor_tensor(
                            out=Rg[:, q, 0:dg, a0:cols].rearrange(
                                "p d (c h) -> p d c h", h=hh
                            ),
                            in0=Rg[:, q, 0:dg, a0:cols].rearrange(
                                "p d (c h) -> p d c h", h=hh
                            ),
                            in1=Rg[:, q, 0:dg, 0:hh][:, :, None, :].to_broadcast(
                                [P, dg, cc, hh]
                            ),
                            op=OP.mult,
                        )
                else:
                    # duplicate ex pairwise so the broadcast stays unit-stride
                    exd = pool.tile([P, GQ, dmax, 2], bf16, tag="exd")
                    nc.vector.tensor_copy(
                        out=exd[:, 0:gq, 0:dg, :],
                        in_=Rg[:, 0:gq, 0:dg, 0:1].to_broadcast([P, gq, dg, 2]),
                    )
                    for q in range(gq):
                        nc.vector.tensor_tensor(
                            out=Rg[:, q, 0:dg, a0:cols].rearrange(
                                "p d (c t) -> p d c t", t=2
                            ),
                            in0=Rg[:, q, 0:dg, a0:cols].rearrange(
                                "p d (c t) -> p d c t", t=2
                            ),
                            in1=exd[:, q, 0:dg, None, :].to_broadcast(
                                [P, dg, cc // 2, 2]
                            ),
                            op=OP.mult,
                        )
                # psum[slot, :] += R[:, q, d, :] for block pairs (N = 2*cols)
                zst = pool.tile([P, GQ, ff], bf16, tag="zst")
                for pi, q0 in enumerate(range(0, gq, 2)):
                    qn = min(2, gq - q0)
                    ps2 = pp.tile([P, 2 * cols], f32, tag=f"pp{pi}")
                    for d in range(dg):
                        nc.tensor.matmul(
                            out=ps2[:, 0 : qn * cols],
                            lhsT=id_t[:],
                            rhs=Rg[:, q0 : q0 + qn, d, :],
                            start=(d == 0),
                            stop=(d == dg - 1),
                        )
                    # epilogue: z = nums / denom  (+ ELU for layer 1)
                    E2 = pool.tile([P, 2, cols], bf16, tag=f"E{pi}")
                    nc.scalar.copy(
                        out=E2[:, 0:qn, :].rearrange("p q f -> p (q f)"),
                        in_=ps2[:, 0 : qn * cols],
                    )
                    rec = pool.tile([P, 2, hh], bf16, tag=f"r{pi}")
                    with nc.allow_low_precision(reason="denom O(1-40)"):
                        nc.vector.reciprocal(
                            out=rec[:, 0:qn, :], in_=E2[:, 0:qn, 0:hh]
                        )
                    zcv = zst[:, q0 : q0 + qn, :]
                    nc.vector.tensor_tensor(
                        out=zcv.rearrange("p q (c h) -> p q c h", h=hh),
                        in0=E2[:, 0:qn, a0:cols].rearrange(
                            "p q (c h) -> p q c h", h=hh
                        ),
                        in1=rec[:, 0:qn, None, :].to_broadcast([P, qn, cc, hh]),
                        op=OP.mult,
                    )
                    if layer == 1:
                        # ELU(x) = max(x, exp(min(x, 0)) - 1)
                        t2 = pool.tile([P, 2, ff], bf16, tag=f"t{pi}")
                        nc.vector.tensor_scalar(
                            out=t2[:, 0:qn, :], in0=zcv, scalar1=0.0,
                            scalar2=None, op0=OP.min,
                        )
                        nc.scalar.activation(
                            out=t2[:, 0:qn, :], in_=t2[:, 0:qn, :], func=AF.Exp
                        )
                        nc.vector.scalar_tensor_tensor(
                            out=zcv, in0=t2[:, 0:qn, :], scalar=-1.0, in1=zcv,
                            op0=OP.add, op1=OP.max,
                        )
                zring = nc.scalar if g % 2 == 0 else nc.sync
                zring.dma_start(
                    out=zout[b0 * P : (b0 + gq) * P, :].rearrange(
                        "(q p) f -> p q f", p=P
                    ),
                    in_=zst[:, 0:gq, :],
                )
                b0 += gq
    nc.finalize()
    return nc


# ------------------------------------------------------------- host pipeline
def _prep(edge_index):
    """Degree-sorted node placement + per-edge gather indices (staging only)."""
    src = np.ascontiguousarray(edge_index[0]).astype(np.int64)
    dst = np.ascontiguousarray(edge_index[1]).astype(np.int64)
    deg = np.bincount(dst, minlength=N) + 1          # + self-loop
    order = np.argsort(-deg, kind="stable")          # rank -> node
    rank = np.empty(N, np.int64)
    rank[order] = np.arange(N)

    dbs = deg[order[0 : N : CHUNK]].astype(np.int64)  # block max degree
    assert len(dbs) == NBLK
    # adaptive groups (<= GQ blocks): uniform degree within each group gives
    # one rectangular DMA per group; small groups at the degree-curve head
    gqs = []
    i = 0
    while i < NBLK:
        k = 1
        while k < GQ and i + k < NBLK:
            pad = (k + 1) * int(dbs[i]) - int(dbs[i : i + k + 1].sum())
            if pad > max(2, (k + 1) * int(dbs[i]) // 25):
                break
            k += 1
        gqs.append(k)
        i += k
    starts = np.cumsum([0] + gqs[:-1])
    dbs = np.repeat(dbs[starts], gqs)
    offk = np.zeros(NBLK + 1, np.int64)
    offk[1:] = np.cumsum(P * dbs)
    tot = int(offk[-1])

    # node -> (core, block, slot)
    k_n = rank // CHUNK
    c_n = (rank % CHUNK) // P
    s_n = rank % P

    gidx = np.full((NCORES, tot), -1, np.int64)
    # self-loops at d = 0
    pos0 = offk[k_n] + s_n * dbs[k_n]
    gidx[c_n, pos0] = np.arange(N)
    # real edges at d = 1.. (order within a node arbitrary)
    o = np.argsort(dst, kind="stable")
    ds, ss = dst[o], src[o]
    estart = np.zeros(N, np.int64)
    estart[1:] = np.cumsum(np.bincount(dst, minlength=N))[:-1]
    d_idx = np.arange(len(ds)) - estart[ds] + 1
    gidx[c_n[ds], offk[k_n[ds]] + s_n[ds] * dbs[k_n[ds]] + d_idx] = ss
    # virtual pad nodes: self-loop to node 0 (output rows dropped)
    vr = np.arange(N, NTOT)
    vk, vc, vs = vr // CHUNK, (vr % CHUNK) // P, vr % P
    gidx[vc, offk[vk] + vs * dbs[vk]] = 0

    # per-core node list in (block, slot) order (for adst / xT / output)
    nodes_of = np.zeros((NCORES, NODE_PAD), np.int64)
    nodes_of[c_n, k_n * P + s_n] = np.arange(N)
    valid = np.zeros((NCORES, NODE_PAD), bool)
    valid[c_n, k_n * P + s_n] = True
    return dbs, gqs, gidx, nodes_of, valid


def _gather(table_ext, gidx_c):
    return np.ascontiguousarray(table_ext[gidx_c + 1])


# column permutation: (h, c) -> c-major (c*H + h)
def _cmajor_perm(hh, cc):
    hcidx = np.arange(hh * cc).reshape(hh, cc)
    return hcidx.T.ravel()


TRACE = False
LAST_EXEC_NS = None
EXEC_TIMES = []
TRACE_DIRS = []


def _ensure_trace_hook():
    import types, importlib

    try:
        import antenv.axon_hooks  # noqa

        return
    except ImportError:
        pass
    import antenv

    mod = types.ModuleType("antenv.axon_hooks")
    _state = {"hook": None}
    mod.set_axon_ntff_profile_hook = lambda h: _state.__setitem__("hook", h)
    mod.get_axon_ntff_profile_hook = lambda: _state["hook"]
    sys.modules["antenv.axon_hooks"] = mod
    antenv.axon_hooks = mod
    if "/root/.axon_site" not in sys.path:
        sys.path.insert(0, "/root/.axon_site")
    tb = importlib.import_module("trn_agent_boot.trn_boot")
    hook = tb._ntff_profile_via_ctypes("/opt/axon/libaxon_pjrt.so")
    mod.set_axon_ntff_profile_hook(hook)


def _run(nc, in_maps):
    global LAST_EXEC_NS
    kw = {}
    if TRACE:
        _ensure_trace_hook()
        import tempfile

        kw = {"trace": True, "tmpdir": tempfile.mkdtemp(prefix="gat_trace_")}
    res = run_bass_kernel_spmd(nc, in_maps, core_ids=list(range(NCORES)), **kw)
    if TRACE:
        TRACE_DIRS.append(kw["tmpdir"])
        if res.exec_time_ns is not None:
            EXEC_TIMES.append(res.exec_time_ns)
            LAST_EXEC_NS = sum(EXEC_TIMES[-4:])
    return res.results


def kernel(
    x,
    edge_index,
    W1,
    att_src1,
    att_dst1,
    bias1,
    W2,
    att_src2,
    att_dst2,
    bias2,
):
    x = np.asarray(x)
    assert np.abs(np.asarray(bias1)).max() == 0.0, "bias1 != 0 unsupported"

    dbs, gqs, gidx, nodes_of, valid = _prep(np.asarray(edge_index))
    dmax = int(dbs.max())
    ident = np.eye(P, dtype=BF)
    perm1 = _cmajor_perm(H1, C1)

    # ---------------- launch A: node stage L1
    w1p = np.asarray(W1)[:, perm1].astype(BF)  # c-major columns
    w1t = np.ascontiguousarray(np.asarray(W1).T).astype(BF)
    atte1 = np.zeros((F1, 2 * H1), np.float32)
    as1, ad1 = np.asarray(att_src1), np.asarray(att_dst1)
    for h in range(H1):
        atte1[h * C1 : (h + 1) * C1, h] = as1[h]
        atte1[h * C1 : (h + 1) * C1, H1 + h] = ad1[h]
    atte1 = atte1.astype(BF)
    xbf = x.astype(BF)
    nc_a = build_node(F1, F1, 2 * H1, NODE_PAD)
    maps_a = [
        {
            "xT": np.ascontiguousarray(xbf[nodes_of[c]].T),
            "w": w1p,
            "wt": w1t,
            "atte": atte1,
        }
        for c in range(NCORES)
    ]
    res_a = _run(nc_a, maps_a)

    # host staging: node-format tables, then per-edge gather
    h1_t = np.zeros((N + 1, F1 + H1), BF)   # [a_src | h]; row 0 = pad
    h1_t[0, 0:H1] = np.float32(NEG)
    ad1_t = np.zeros((N, H1), BF)
    for c in range(NCORES):
        outT = res_a[c]["outT"]             # [144, 12544] bf16
        v = valid[c]
        nds = nodes_of[c][v]
        h1_t[nds + 1, H1 : H1 + F1] = outT[0:F1, v].T
        h1_t[nds + 1, 0:H1] = outT[F1 : F1 + H1, v].T
        ad1_t[nds] = outT[F1 + H1 : F1 + 2 * H1, v].T

    # ---------------- launch B: edge stage L1
    nc_b = build_edge(1, dbs, gqs, dmax)
    maps_b = [
        {
            "hgat": _gather(h1_t, gidx[c]),
            "adst": np.ascontiguousarray(ad1_t[nodes_of[c]]),
            "ident": ident,
        }
        for c in range(NCORES)
    ]
    res_b = _run(nc_b, maps_b)

    # ---------------- launch C: node stage L2
    w2p = np.asarray(W2)[perm1, :].astype(BF)  # rows permuted to c-major z1
    w2t = np.ascontiguousarray(w2p.T)
    att2 = np.stack(
        [np.asarray(att_src2).ravel(), np.asarray(att_dst2).ravel()], axis=1
    ).astype(BF)
    nc_c = build_node(F1, F2, 2, NODE_PAD)
    maps_c = [
        {
            "xT": np.ascontiguousarray(res_b[c]["z"].T),
            "w": w2p,
            "wt": w2t,
            "atte": att2,
        }
        for c in range(NCORES)
    ]
    res_c = _run(nc_c, maps_c)

    h2_t = np.zeros((N + 1, 66), BF)        # [a_src2 | pad | h2]; row 0 = pad
    h2_t[0, 0] = np.float32(NEG)
    ad2_t = np.zeros((N, 1), BF)
    for c in range(NCORES):
        outT = res_c[c]["outT"]             # [66, 12544]
        v = valid[c]
        nds = nodes_of[c][v]
        h2_t[nds + 1, 2:66] = outT[0:F2, v].T
        h2_t[nds + 1, 0] = outT[F2, v]
        ad2_t[nds, 0] = outT[F2 + 1, v]

    # ---------------- launch D: edge stage L2
    nc_d = build_edge(2, dbs, gqs, dmax)
    maps_d = [
        {
            "hgat": _gather(h2_t, gidx[c]),
            "adst": np.ascontiguousarray(ad2_t[nodes_of[c]]),
            "ident": ident,
        }
        for c in range(NCORES)
    ]
    res_d = _run(nc_d, maps_d)

    out = np.zeros((N, F2), np.float32)
    for c in range(NCORES):
        v = valid[c]
        out[nodes_of[c][v]] = res_d[c]["z"][v].astype(np.float32)
    return out + np.asarray(bias2)[None, :].astype(np.float32)


# revision 35
# speedup vs baseline: 1.2269x; 1.2269x over previous
"""GAT (2-layer) on 8 Trainium2 NeuronCores — streaming edge aggregation.

Strategy (graph/data parallel per the sharding hint, node-partitioned):
- Host-side staging only (index gathers / permutation / padding); ALL
  arithmetic stays on device.  Nodes are sorted by in-degree and dealt
  into 98 blocks x 1024 (128 per core x 8 cores), so each block has a
  near-uniform degree D_b (max over its 1024 nodes); per-node edge lists
  (self-loop first) are padded to D_b.
- Edge stage: per-edge source features h[src] and a_src[src] are gathered
  by the host into a contiguous slot-major stream [block][slot][d][cols]
  (pure data movement of device-computed values, like the baseline's
  _expand_a), so the device only does big sequential HWDGE DMAs — no
  dma_gather / GpSimd descriptor generation at all.
- Segment softmax+sum per block of 128 dst nodes: partition = dst slot;
  R[slot, d, :] = [ex | ex * h] after DVE add / leaky / ActE exp / DVE
  mult; then D_b identity-stationary matmuls accumulate
  psum[slot, [denom | out]] += R[:, d, :] (TensorE as a wide fp32
  accumulator; identity weights loaded once).  Softmax max-subtraction
  is skipped (logits are O(1), exp is safe).
- Node stages: weights stationary (Wcat = [W | W @ atte] built on device
  with one matmul), x^T streamed in 512-column chunks; host transposes
  between launches, so no on-device transposes anywhere.
- Head dim c-major (col = c*H + h) so the ex*h broadcast multiply is
  unit-stride innermost (DVE 2x).  bf16 storage/compute, fp32 PSUM.
- ELU(x) = max(x, exp(min(x, 0)) - 1)  (1 DVE min, 1 ActE exp, 1 DVE stt).
"""

import sys

sys.path.insert(0, "/opt/trn_rl_repo")

import numpy as np
import ml_dtypes

import concourse.bass as bass
import concourse.mybir as mybir
from concourse import bacc
from concourse.tile import TileContext
from concourse.bass_utils import run_bass_kernel_spmd

BF = ml_dtypes.bfloat16
bf16 = mybir.dt.bfloat16
f32 = mybir.dt.float32
AF = mybir.ActivationFunctionType
OP = mybir.AluOpType

N = 100000
NCORES = 8
P = 128
NBLK = 98                 # blocks per core; 98*128 = 12544 nodes/core
NODE_PAD = NBLK * P       # 12544
NTOT = NODE_PAD * NCORES  # 100352
CHUNK = 1024              # nodes per block across all cores (128 * 8)
H1, C1, F1 = 8, 16, 128   # layer-1 heads/channels; F1 = H1*C1
F2 = 64
NEG = -60000.0
GQ = 7                    # blocks per DMA group (98 = 14*7); uniform degree
NG = NBLK // GQ           # 14 groups


# ---------------------------------------------------------------- node stage
def build_node(fin, fout, extra, nodes):
    """outT = [Wcat^T @ xT] rows: [h (fout) | att terms (extra)].

    xT: [fin, nodes] shard; Wcat = [W | W @ atte] built on device.
    """
    nc = bacc.Bacc(trn_type="TRN2")
    xT = nc.declare_dram_parameter("xT", [fin, nodes], bf16, isOutput=False)
    w = nc.declare_dram_parameter("w", [fin, fout], bf16, isOutput=False)
    wt = nc.declare_dram_parameter("wt", [fout, fin], bf16, isOutput=False)
    atte = nc.declare_dram_parameter("atte", [fout, extra], bf16, isOutput=False)
    outT = nc.declare_dram_parameter(
        "outT", [fout + extra, nodes], bf16, isOutput=True
    )

    nch = (nodes + 511) // 512
    with TileContext(nc) as tc:
        with (
            tc.tile_pool(name="const", bufs=1) as cp,
            tc.tile_pool(name="sb", bufs=2) as pool,
            tc.tile_pool(name="ps", bufs=2, space="PSUM") as pp,
        ):
            # PE warmup: ~4us of dummy matmuls releases the HAM clock gate
            # (1.2 -> 2.4 GHz) while the input DMA streams in
            zt = cp.tile([P, P], bf16)
            nc.scalar.memzero(zt[:])
            wu_ps = pp.tile([P, P], f32, tag="wu")
            for _ in range(40):
                nc.tensor.matmul(
                    out=wu_ps[:], lhsT=zt[:], rhs=zt[:], start=True, stop=True
                )
            wcat = cp.tile([fin, fout + extra], bf16)
            nc.sync.dma_start(out=wcat[:, 0:fout], in_=w[:])
            wt_t = cp.tile([fout, fin], bf16)
            nc.sync.dma_start(out=wt_t[:], in_=wt[:])
            atte_t = cp.tile([fout, extra], bf16)
            nc.sync.dma_start(out=atte_t[:], in_=atte[:])
            # w_att[fi, e] = sum_hc W[fi, hc] * atte[hc, e]
            wa_ps = pp.tile([fin, extra], f32, tag="wa")
            nc.tensor.matmul(
                out=wa_ps[:], lhsT=wt_t[:], rhs=atte_t[:], start=True, stop=True
            )
            nc.vector.tensor_copy(out=wcat[:, fout : fout + extra], in_=wa_ps[:])

            xs = cp.tile([fin, nodes], bf16)
            for c0 in range(0, nodes, 4096):
                c1 = min(nodes, c0 + 4096)
                nc.sync.dma_start(out=xs[:, c0:c1], in_=xT[:, c0:c1])
            # output in quarters so the store DMAs overlap the matmuls
            QW = 3584
            qbounds = list(range(0, nodes, QW)) + [nodes]
            hqt = [
                cp.tile([fout, min(QW, nodes - q0)], bf16, name=f"hq{qi}")
                for qi, q0 in enumerate(qbounds[:-1])
            ]
            aqt = [
                cp.tile([extra, min(QW, nodes - q0)], bf16, name=f"aq{qi}")
                for qi, q0 in enumerate(qbounds[:-1])
            ]
            for i in range(nch):
                c0 = i * 512
                c1 = min(nodes, c0 + 512)
                qi = c0 // QW
                l0 = c0 - qi * QW
                h_ps = pp.tile([fout, 512], f32, tag="h")
                nc.tensor.matmul(
                    out=h_ps[:, 0 : c1 - c0],
                    lhsT=wcat[:, 0:fout],
                    rhs=xs[:, c0:c1],
                    start=True,
                    stop=True,
                )
                a_ps = pp.tile([extra, 512], f32, tag="a")
                nc.tensor.matmul(
                    out=a_ps[:, 0 : c1 - c0],
                    lhsT=wcat[:, fout : fout + extra],
                    rhs=xs[:, c0:c1],
                    start=True,
                    stop=True,
                )
                nc.scalar.copy(
                    out=hqt[qi][:, l0 : l0 + c1 - c0], in_=h_ps[:, 0 : c1 - c0]
                )
                nc.vector.tensor_copy(
                    out=aqt[qi][:, l0 : l0 + c1 - c0], in_=a_ps[:, 0 : c1 - c0]
                )
                if c1 == qbounds[qi + 1]:  # quarter complete -> store it
                    q0 = qi * QW
                    nc.sync.dma_start(
                        out=outT[0:fout, q0:c1], in_=hqt[qi][:]
                    )
                    nc.scalar.dma_start(
                        out=outT[fout : fout + extra, q0:c1], in_=aqt[qi][:]
                    )
    nc.finalize()
    return nc


# ---------------------------------------------------------------- edge stage
def build_edge(layer, dbs, gqs, dmax, fuse2=False):
    """Edge aggregation over the core's 98 dst blocks (dbs[b] = edges/node).

    dbs is uniform within each group of GQ blocks; one DMA per group.
    Blocks are aggregated in pairs via N=2*cols matmuls into one PSUM bank.
    With fuse2, the layer-2 node stage runs in the same launch: each block's
    z is PE-transposed into z1T and the kernel emits outT2 = Wcat2^T @ z1T
    instead of z itself.
    """
    if layer == 1:
        hh, cc, ff = H1, C1, F1      # cols: [a_src(8) | h(128)] -> 136
        cols = hh + ff
        a0 = hh                       # h starts at col 8 (16B aligned)
    else:
        hh, cc, ff = 1, F2, F2        # cols: [a_src(1) | pad(1) | h(64)] -> 66
        cols = 66
        a0 = 2
    totrows = P * int(sum(dbs))

    nc = bacc.Bacc(trn_type="TRN2")
    hgat = nc.declare_dram_parameter("hgat", [totrows, cols], bf16, isOutput=False)
    adst = nc.declare_dram_parameter("adst", [NBLK * P, hh], bf16, isOutput=False)
    ident = nc.declare_dram_parameter("ident", [P, P], bf16, isOutput=False)
    if fuse2:
        w2 = nc.declare_dram_parameter("w2", [F1, F2], bf16, isOutput=False)
        w2t = nc.declare_dram_parameter("w2t", [F2, F1], bf16, isOutput=False)
        atte2 = nc.declare_dram_parameter("atte2", [F2, 2], bf16, isOutput=False)
        outT2 = nc.declare_dram_parameter(
            "outT2", [F2 + 2, NBLK * P], bf16, isOutput=True
        )
    else:
        zout = nc.declare_dram_parameter("z", [NBLK * P, ff], bf16, isOutput=True)

    # tile shapes: group 0 (gq=1) carries the degree-distribution head; all
    # other groups have much smaller dg, so give them a separate (3-deep) tag
    dmax2 = max(
        (int(dbs[b]) for b, q in zip(np.cumsum([0] + list(gqs[:-1])), gqs)
         if q > 1),
        default=dmax,
    )
    with TileContext(nc) as tc:
        with (
            tc.tile_pool(name="const", bufs=1) as cp,
            tc.tile_pool(name="sbb", bufs=1) as bpool,
            tc.tile_pool(name="sb", bufs=3) as gpool,
            tc.tile_pool(name="ep", bufs=2) as pool,
            tc.tile_pool(name="ps", bufs=2, space="PSUM") as pp,
        ):
            id_t = cp.tile([P, P], bf16)
            nc.sync.dma_start(out=id_t[:], in_=ident[:])
            ad_t = cp.tile([P, NBLK, hh], bf16)
            nc.sync.dma_start(
                out=ad_t[:], in_=adst[:].rearrange("(b p) h -> p b h", p=P)
            )
            if fuse2:
                wcat2 = cp.tile([F1, F2 + 2], bf16)
                nc.sync.dma_start(out=wcat2[:, 0:F2], in_=w2[:])
                w2t_t = cp.tile([F2, F1], bf16)
                nc.sync.dma_start(out=w2t_t[:], in_=w2t[:])
                atte2_t = cp.tile([F2, 2], bf16)
                nc.sync.dma_start(out=atte2_t[:], in_=atte2[:])
                wa2_ps = pp.tile([F1, 2], f32, tag="wa2", bufs=1)
                nc.tensor.matmul(
                    out=wa2_ps[:], lhsT=w2t_t[:], rhs=atte2_t[:],
                    start=True, stop=True,
                )
                nc.vector.tensor_copy(
                    out=wcat2[:, F2 : F2 + 2], in_=wa2_ps[:]
                )
                o2T = cp.tile([F2 + 2, NBLK * P], bf16)

            off = 0
            b0 = 0
            for g, gq in enumerate(gqs):
                dg = int(dbs[b0])
                if gq == 1:
                    Rg = bpool.tile([P, 1, dmax, cols], bf16, tag="Rb")
                else:
                    Rg = gpool.tile([P, GQ, dmax2, cols], bf16, tag="R")
                nc.sync.dma_start(
                    out=Rg[:, 0:gq, 0:dg, :],
                    in_=hgat[off : off + gq * P * dg, :].rearrange(
                        "(q p d) f -> p q d f", q=gq, p=P
                    ),
                )
                off += gq * P * dg
                # e = a_src + a_dst ; leaky_relu(0.2) ; exp
                nc.vector.tensor_tensor(
                    out=Rg[:, 0:gq, 0:dg, 0:hh],
                    in0=Rg[:, 0:gq, 0:dg, 0:hh],
                    in1=ad_t[:, b0 : b0 + gq, None, :].to_broadcast(
                        [P, gq, dg, hh]
                    ),
                    op=OP.add,
                )
                lkf = pool.tile([P, GQ * dmax2 * hh], bf16, tag="lk", bufs=1)
                lk = lkf[:, 0 : gq * dg * hh].rearrange(
                    "p (q d h) -> p q d h", q=gq, d=dg
                )
                nc.vector.tensor_scalar(
                    out=lk, in0=Rg[:, 0:gq, 0:dg, 0:hh],
                    scalar1=0.2, scalar2=None, op0=OP.mult,
                )
                nc.vector.tensor_tensor(
                    out=Rg[:, 0:gq, 0:dg, 0:hh],
                    in0=Rg[:, 0:gq, 0:dg, 0:hh],
                    in1=lk,
                    op=OP.max,
                )
                nc.scalar.activation(
                    out=Rg[:, 0:gq, 0:dg, 0:hh],
                    in_=Rg[:, 0:gq, 0:dg, 0:hh],
                    func=AF.Exp,
                )
                # msg = ex * h  (unit-stride innermost on both operands;
                # per block: the ISA mem pattern allows at most 3 free dims)
                if layer == 1:
                    for q in range(gq):
                        nc.vector.tensor_tensor(
                            out=Rg[:, q, 0:dg, a0:cols].rearrange(
                                "p d (c h) -> p d c h", h=hh
                            ),
                            in0=Rg[:, q, 0:dg, a0:cols].rearrange(
                                "p d (c h) -> p d c h", h=hh
                            ),
                            in1=Rg[:, q, 0:dg, 0:hh][:, :, None, :].to_broadcast(
                                [P, dg, cc, hh]
                            ),
                            op=OP.mult,
                        )
                else:
                    # duplicate ex pairwise so the broadcast stays unit-stride
                    exd = pool.tile([P, GQ, dmax, 2], bf16, tag="exd")
                    nc.vector.tensor_copy(
                        out=exd[:, 0:gq, 0:dg, :],
                        in_=Rg[:, 0:gq, 0:dg, 0:1].to_broadcast([P, gq, dg, 2]),
                    )
                    for q in range(gq):
                        nc.vector.tensor_tensor(
                            out=Rg[:, q, 0:dg, a0:cols].rearrange(
                                "p d (c t) -> p d c t", t=2
                            ),
                            in0=Rg[:, q, 0:dg, a0:cols].rearrange(
                                "p d (c t) -> p d c t", t=2
                            ),
                            in1=exd[:, q, 0:dg, None, :].to_broadcast(
                                [P, dg, cc // 2, 2]
                            ),
                            op=OP.mult,
                        )
                # psum[slot, :] += R[:, q, d, :] for block pairs (N = 2*cols)
                zst = pool.tile([P, GQ, ff], bf16, tag="zst")
                for pi, q0 in enumerate(range(0, gq, 2)):
                    qn = min(2, gq - q0)
                    ps2 = pp.tile([P, 2 * cols], f32, tag=f"pp{pi % 2}")
                    for d in range(dg):
                        nc.tensor.matmul(
                            out=ps2[:, 0 : qn * cols],
                            lhsT=id_t[:],
                            rhs=Rg[:, q0 : q0 + qn, d, :],
                            start=(d == 0),
                            stop=(d == dg - 1),
                        )
                    # epilogue: z = nums / denom  (+ ELU for layer 1)
                    E2 = pool.tile([P, 2, cols], bf16, tag=f"E{pi}")
                    nc.scalar.copy(
                        out=E2[:, 0:qn, :].rearrange("p q f -> p (q f)"),
                        in_=ps2[:, 0 : qn * cols],
                    )
                    rec = pool.tile([P, 2, hh], bf16, tag=f"r{pi}")
                    with nc.allow_low_precision(reason="denom O(1-40)"):
                        nc.vector.reciprocal(
                            out=rec[:, 0:qn, :], in_=E2[:, 0:qn, 0:hh]
                        )
                    zcv = zst[:, q0 : q0 + qn, :]
                    nc.vector.tensor_tensor(
                        out=zcv.rearrange("p q (c h) -> p q c h", h=hh),
                        in0=E2[:, 0:qn, a0:cols].rearrange(
                            "p q (c h) -> p q c h", h=hh
                        ),
                        in1=rec[:, 0:qn, None, :].to_broadcast([P, qn, cc, hh]),
                        op=OP.mult,
                    )
                    if layer == 1:
                        # ELU(x) = max(x, exp(min(x, 0)) - 1)
                        t2 = pool.tile([P, 2, ff], bf16, tag=f"t{pi}")
                        nc.vector.tensor_scalar(
                            out=t2[:, 0:qn, :], in0=zcv, scalar1=0.0,
                            scalar2=None, op0=OP.min,
                        )
                        nc.scalar.activation(
                            out=t2[:, 0:qn, :], in_=t2[:, 0:qn, :], func=AF.Exp
                        )
                        nc.vector.scalar_tensor_tensor(
                            out=zcv, in0=t2[:, 0:qn, :], scalar=-1.0, in1=zcv,
                            op0=OP.add, op1=OP.max,
                        )
                if fuse2:
                    # layer-2 node stage inline: transpose z blocks, then
                    # o2T[:, group cols] = Wcat2^T @ z_group^T
                    zqT = pool.tile([F1, GQ * P], bf16, tag="zqT")
                    for q in range(gq):
                        tr_ps = pp.tile([P, P], bf16, tag="tr", bufs=1)
                        nc.tensor.transpose(
                            out=tr_ps[:], in_=zst[:, q, :], identity=id_t[:]
                        )
                        nc.scalar.copy(
                            out=zqT[:, q * P : (q + 1) * P], in_=tr_ps[:]
                        )
                    gw = gq * P
                    for l0 in range(0, gw, 512):
                        l1 = min(gw, l0 + 512)
                        h2_ps = pp.tile([F2 + 2, 512], f32, tag="h2")
                        nc.tensor.matmul(
                            out=h2_ps[:, 0 : l1 - l0],
                            lhsT=wcat2[:],
                            rhs=zqT[:, l0:l1],
                            start=True,
                            stop=True,
                        )
                        nc.scalar.copy(
                            out=o2T[:, b0 * P + l0 : b0 * P + l1],
                            in_=h2_ps[:, 0 : l1 - l0],
                        )
                else:
                    nc.scalar.dma_start(
                        out=zout[b0 * P : (b0 + gq) * P, :].rearrange(
                            "(q p) f -> p q f", p=P
                        ),
                        in_=zst[:, 0:gq, :],
                    )
                b0 += gq

            if fuse2:
                nc.sync.dma_start(out=outT2[:], in_=o2T[:])
    nc.finalize()
    return nc


# ------------------------------------------------------------- host pipeline
def _prep(edge_index):
    """Degree-sorted node placement + per-edge gather indices (staging only)."""
    src = np.ascontiguousarray(edge_index[0]).astype(np.int64)
    dst = np.ascontiguousarray(edge_index[1]).astype(np.int64)
    deg = np.bincount(dst, minlength=N) + 1          # + self-loop
    order = np.argsort(-deg, kind="stable")          # rank -> node
    rank = np.empty(N, np.int64)
    rank[order] = np.arange(N)

    dbs = deg[order[0 : N : CHUNK]].astype(np.int64)  # block max degree
    assert len(dbs) == NBLK
    # adaptive groups (<= GQ blocks): uniform degree within each group gives
    # one rectangular DMA per group; small groups at the degree-curve head
    gqs = []
    i = 0
    while i < NBLK:
        k = 1
        while k < GQ and i + k < NBLK:
            pad = (k + 1) * int(dbs[i]) - int(dbs[i : i + k + 1].sum())
            if pad > max(2, (k + 1) * int(dbs[i]) // 25):
                break
            k += 1
        gqs.append(k)
        i += k
    starts = np.cumsum([0] + gqs[:-1])
    dbs = np.repeat(dbs[starts], gqs)
    offk = np.zeros(NBLK + 1, np.int64)
    offk[1:] = np.cumsum(P * dbs)
    tot = int(offk[-1])

    # node -> (core, block, slot)
    k_n = rank // CHUNK
    c_n = (rank % CHUNK) // P
    s_n = rank % P

    gidx = np.full((NCORES, tot), -1, np.int64)
    # self-loops at d = 0
    pos0 = offk[k_n] + s_n * dbs[k_n]
    gidx[c_n, pos0] = np.arange(N)
    # real edges at d = 1.. (order within a node arbitrary)
    o = np.argsort(dst, kind="stable")
    ds, ss = dst[o], src[o]
    estart = np.zeros(N, np.int64)
    estart[1:] = np.cumsum(np.bincount(dst, minlength=N))[:-1]
    d_idx = np.arange(len(ds)) - estart[ds] + 1
    gidx[c_n[ds], offk[k_n[ds]] + s_n[ds] * dbs[k_n[ds]] + d_idx] = ss
    # virtual pad nodes: self-loop to node 0 (output rows dropped)
    vr = np.arange(N, NTOT)
    vk, vc, vs = vr // CHUNK, (vr % CHUNK) // P, vr % P
    gidx[vc, offk[vk] + vs * dbs[vk]] = 0

    # per-core node list in (block, slot) order (for adst / xT / output)
    nodes_of = np.zeros((NCORES, NODE_PAD), np.int64)
    nodes_of[c_n, k_n * P + s_n] = np.arange(N)
    valid = np.zeros((NCORES, NODE_PAD), bool)
    valid[c_n, k_n * P + s_n] = True
    return dbs, gqs, gidx, nodes_of, valid


def _gather(table_ext, gidx_c):
    return np.ascontiguousarray(table_ext[gidx_c + 1])


# column permutation: (h, c) -> c-major (c*H + h)
def _cmajor_perm(hh, cc):
    hcidx = np.arange(hh * cc).reshape(hh, cc)
    return hcidx.T.ravel()


TRACE = False
LAST_EXEC_NS = None
EXEC_TIMES = []
TRACE_DIRS = []


def _ensure_trace_hook():
    import types, importlib

    try:
        import antenv.axon_hooks  # noqa

        return
    except ImportError:
        pass
    import antenv

    mod = types.ModuleType("antenv.axon_hooks")
    _state = {"hook": None}
    mod.set_axon_ntff_profile_hook = lambda h: _state.__setitem__("hook", h)
    mod.get_axon_ntff_profile_hook = lambda: _state["hook"]
    sys.modules["antenv.axon_hooks"] = mod
    antenv.axon_hooks = mod
    if "/root/.axon_site" not in sys.path:
        sys.path.insert(0, "/root/.axon_site")
    tb = importlib.import_module("trn_agent_boot.trn_boot")
    hook = tb._ntff_profile_via_ctypes("/opt/axon/libaxon_pjrt.so")
    mod.set_axon_ntff_profile_hook(hook)


def _run(nc, in_maps):
    global LAST_EXEC_NS
    kw = {}
    if TRACE:
        _ensure_trace_hook()
        import tempfile

        kw = {"trace": True, "tmpdir": tempfile.mkdtemp(prefix="gat_trace_")}
    res = run_bass_kernel_spmd(nc, in_maps, core_ids=list(range(NCORES)), **kw)
    if TRACE:
        TRACE_DIRS.append(kw["tmpdir"])
        if res.exec_time_ns is not None:
            EXEC_TIMES.append(res.exec_time_ns)
            LAST_EXEC_NS = sum(EXEC_TIMES[-4:])
    return res.results


def kernel(
    x,
    edge_index,
    W1,
    att_src1,
    att_dst1,
    bias1,
    W2,
    att_src2,
    att_dst2,
    bias2,
):
    x = np.asarray(x)
    assert np.abs(np.asarray(bias1)).max() == 0.0, "bias1 != 0 unsupported"

    dbs, gqs, gidx, nodes_of, valid = _prep(np.asarray(edge_index))
    dmax = int(dbs.max())
    ident = np.eye(P, dtype=BF)
    perm1 = _cmajor_perm(H1, C1)

    # ---------------- launch A: node stage L1
    w1p = np.asarray(W1)[:, perm1].astype(BF)  # c-major columns
    w1t = np.ascontiguousarray(np.asarray(W1).T).astype(BF)
    atte1 = np.zeros((F1, 2 * H1), np.float32)
    as1, ad1 = np.asarray(att_src1), np.asarray(att_dst1)
    for h in range(H1):
        atte1[h * C1 : (h + 1) * C1, h] = as1[h]
        atte1[h * C1 : (h + 1) * C1, H1 + h] = ad1[h]
    atte1 = atte1.astype(BF)
    xbf = x.astype(BF)
    nc_a = build_node(F1, F1, 2 * H1, NODE_PAD)
    maps_a = [
        {
            "xT": np.ascontiguousarray(xbf[nodes_of[c]].T),
            "w": w1p,
            "wt": w1t,
            "atte": atte1,
        }
        for c in range(NCORES)
    ]
    res_a = _run(nc_a, maps_a)

    # host staging: node-format tables, then per-edge gather
    h1_t = np.zeros((N + 1, F1 + H1), BF)   # [a_src | h]; row 0 = pad
    h1_t[0, 0:H1] = np.float32(NEG)
    ad1_t = np.zeros((N, H1), BF)
    for c in range(NCORES):
        outT = res_a[c]["outT"]             # [144, 12544] bf16
        v = valid[c]
        nds = nodes_of[c][v]
        h1_t[nds + 1, H1 : H1 + F1] = outT[0:F1, v].T
        h1_t[nds + 1, 0:H1] = outT[F1 : F1 + H1, v].T
        ad1_t[nds] = outT[F1 + H1 : F1 + 2 * H1, v].T

    # ---------------- launch B: edge stage L1 + node stage L2 (fused)
    w2p = np.asarray(W2)[perm1, :].astype(BF)  # rows permuted to c-major z1
    w2t = np.ascontiguousarray(w2p.T)
    att2 = np.stack(
        [np.asarray(att_src2).ravel(), np.asarray(att_dst2).ravel()], axis=1
    ).astype(BF)
    nc_b = build_edge(1, dbs, gqs, dmax, fuse2=True)
    maps_b = [
        {
            "hgat": _gather(h1_t, gidx[c]),
            "adst": np.ascontiguousarray(ad1_t[nodes_of[c]]),
            "ident": ident,
            "w2": w2p,
            "w2t": w2t,
            "atte2": att2,
        }
        for c in range(NCORES)
    ]
    res_b = _run(nc_b, maps_b)

    h2_t = np.zeros((N + 1, 66), BF)        # [a_src2 | pad | h2]; row 0 = pad
    h2_t[0, 0] = np.float32(NEG)
    ad2_t = np.zeros((N, 1), BF)
    for c in range(NCORES):
        outT = res_b[c]["outT2"]            # [66, 12544]
        v = valid[c]
        nds = nodes_of[c][v]
        h2_t[nds + 1, 2:66] = outT[0:F2, v].T
        h2_t[nds + 1, 0] = outT[F2, v]
        ad2_t[nds, 0] = outT[F2 + 1, v]

    # ---------------- launch D: edge stage L2
    nc_d = build_edge(2, dbs, gqs, dmax)
    maps_d = [
        {
            "hgat": _gather(h2_t, gidx[c]),
            "adst": np.ascontiguousarray(ad2_t[nodes_of[c]]),
            "ident": ident,
        }
        for c in range(NCORES)
    ]
    res_d = _run(nc_d, maps_d)

    out = np.zeros((N, F2), np.float32)
    for c in range(NCORES):
        v = valid[c]
        out[nodes_of[c][v]] = res_d[c]["z"][v].astype(np.float32)
    return out + np.asarray(bias2)[None, :].astype(np.float32)
